# revision 23
# baseline (speedup 1.0000x reference)
"""Multi-head attention Trainium2 kernel (nn_MultiHeadAttention_489626272000).

Strategy
--------
Data-parallel over batch: 32 batches -> 8 NeuronCores, 4 batches/core.

Per batch, everything is computed in a "transposed score" orientation
S^T[t, q] so that
  * the score matmul S^T = K_h^T-major is a K=16 contraction packed 4 heads
    at a time into PE row-groups (tile_position=(32g, 0)),
  * exp() runs on ScalarE straight out of PSUM (4 heads = 4 banks = one
    N=2048 ACTIVATE),
  * the mask is a plain elementwise multiply on VectorE (bf16, 2x mode),
  * the AV matmul consumes Ẽ^T directly (contraction over t = partitions),
    packed 4 heads into PE col-groups (tile_position=(0, 32g)), with a
    ones-column appended to V so the softmax denominator falls out of the
    same matmul,
  * softmax normalization happens once per [hv=128, q] tile at the end
    (exact-f32 sums via partition-gather DMAs + one VectorE reciprocal).

Host-side prep (inside kernel()): transpose q/h/mask per-batch and cast to
bf16; pack weights into PE-friendly group layouts. The device kernel then
needs zero on-chip transposes.

No max-subtraction is needed before exp: scores have sigma≈2.7, so the max
over all 268M scores is ~15 -> exp() <= ~3e6, far inside fp32/bf16 range.
"""

import sys

for _p in ("/opt/trn_rl_repo",):
    if _p not in sys.path:
        sys.path.insert(0, _p)

from contextlib import ExitStack

import ml_dtypes
import numpy as np

import concourse.bass as bass
import concourse.tile as tile
from concourse import bacc, mybir
from concourse.bass_utils import run_bass_kernel_spmd

BF16 = ml_dtypes.bfloat16
F32 = np.float32

N_CORES = 8
B, NQ, T, EMB = 32, 1024, 1024, 128
H, DK = 8, 16
BL = B // N_CORES  # batches per core
SCALE = 1.0 / np.sqrt(DK)  # folded into w_query on the host

TT = T // 128  # 8 t-tiles per batch
QC = NQ // 512  # 2 q-chunks per batch
DT = mybir.dt


def _build_kernel(repeat=1, score_dt=None):
    nc = bacc.Bacc("TRN2", target_bir_lowering=False, debug=False)

    qT = nc.dram_tensor("qt", [BL, EMB, NQ], DT.float32r, kind="ExternalInput")
    hT = nc.dram_tensor("ht", [BL, EMB, T], DT.float32r, kind="ExternalInput")
    mT = nc.dram_tensor("mt", [BL, T, NQ], DT.bfloat16, kind="ExternalInput")
    # wq/wk: [emb, grp, 128] where col 32*g+v holds head (4*grp+g) dim v
    # (cols 32g+16..32g+32 are zero padding so heads align to PE row groups)
    wq = nc.dram_tensor("wq", [EMB, 2, 128], DT.float32r, kind="ExternalInput")
    wk = nc.dram_tensor("wk", [EMB, 2, 128], DT.float32r, kind="ExternalInput")
    # wv: [emb, h*16+v] plain packing; wo: [h*16+v, emb]
    wv = nc.dram_tensor("wv", [EMB, 128], DT.float32r, kind="ExternalInput")
    wo = nc.dram_tensor("wo", [128, EMB], DT.float32r, kind="ExternalInput")
    # sums-replication selector: s_rep[64*rnd + 16g + v, q] = av_rnd[32g + 16, q]
    sels = nc.dram_tensor("sels", [2, 128, 64], DT.float32, kind="ExternalInput")
    out = nc.dram_tensor("out", [BL, NQ, EMB], DT.float32, kind="ExternalOutput")

    if score_dt is None:
        score_dt = DT.float32r
    with tile.TileContext(nc) as tc, ExitStack() as ctx:
        # ---- pools ----
        singles = ctx.enter_context(tc.tile_pool(name="singles", bufs=1))
        inp = ctx.enter_context(tc.tile_pool(name="inp", bufs=2))
        qk = ctx.enter_context(tc.tile_pool(name="qk", bufs=8))
        vpool = ctx.enter_context(tc.tile_pool(name="v", bufs=16))
        epool = ctx.enter_context(tc.tile_pool(name="e", bufs=3))
        empool = ctx.enter_context(tc.tile_pool(name="em", bufs=3))
        epi = ctx.enter_context(tc.tile_pool(name="epi", bufs=2))
        opool = ctx.enter_context(tc.tile_pool(name="o", bufs=4))

        ps_s_pool = ctx.enter_context(tc.tile_pool(name="pss", bufs=1, space="PSUM"))
        ps_av_pool = ctx.enter_context(tc.tile_pool(name="psav", bufs=2, space="PSUM"))
        ps_sm_pool = ctx.enter_context(tc.tile_pool(name="pssm", bufs=2, space="PSUM"))

        # ---- constants ----
        wq_sb = singles.tile([EMB, 2, 128], DT.float32r)
        nc.sync.dma_start(wq_sb, wq[:])
        wk_sb = singles.tile([EMB, 2, 128], DT.float32r)
        nc.sync.dma_start(wk_sb, wk[:])
        wv_sb = singles.tile([EMB, 128], DT.float32r)
        nc.sync.dma_start(wv_sb, wv[:])
        wo_sb = singles.tile([128, EMB], DT.float32r)
        nc.sync.dma_start(wo_sb, wo[:])
        sels_sb = singles.tile([128, 2, 64], DT.float32)
        nc.sync.dma_start(sels_sb, sels[:].rearrange("r p m -> p r m"))

        def emit_load(b):
            qt_in = inp.tile([128, NQ], DT.float32r, tag="qt", name="qt_in")
            nc.sync.dma_start(qt_in, qT[b])
            ht_in = inp.tile([128, T], DT.float32r, tag="ht", name="ht_in")
            nc.sync.dma_start(ht_in, hT[b])
            m_in = inp.tile([128, TT, NQ], DT.bfloat16, tag="mt", name="m_in")
            nc.sync.dma_start(m_in, mT[b].rearrange("(tt p) q -> p tt q", p=128))
            return qt_in, ht_in, m_in

        def emit_proj(loaded):
            qt_in, ht_in, m_in = loaded
            qt_g, kt_g = [], []
            for grp in range(2):
                qg = qk.tile([128, NQ], DT.bfloat16, tag="qk", name="qg")
                qe = qk.tile([128, NQ], DT.bfloat16, tag="qk", name="qe")
                kg = qk.tile([128, T], DT.bfloat16, tag="qk", name="kg")
                ke = qk.tile([128, T], DT.bfloat16, tag="qk", name="ke")
                for half in range(2):
                    sl = bass.ts(half, 512)
                    psq = ps_sm_pool.tile([128, 512], DT.float32, tag="sm", name="psq")
                    nc.tensor.matmul(
                        psq,
                        lhsT=wq_sb[:, grp, :],
                        rhs=qt_in[:, sl],
                        start=True, stop=True,
                    )
                    nc.vector.tensor_copy(qg[:, sl], psq)
                    nc.vector.tensor_tensor(
                        qe[:, sl], psq, qg[:, sl], mybir.AluOpType.subtract
                    )
                    psk = ps_sm_pool.tile([128, 512], DT.float32, tag="sm", name="psk")
                    nc.tensor.matmul(
                        psk,
                        lhsT=wk_sb[:, grp, :],
                        rhs=ht_in[:, sl],
                        start=True, stop=True,
                    )
                    nc.vector.tensor_copy(kg[:, sl], psk)
                    nc.vector.tensor_tensor(
                        ke[:, sl], psk, kg[:, sl], mybir.AluOpType.subtract
                    )
                qt_g.append((qg, qe))
                kt_g.append((kg, ke))

            v_tiles = []
            for tt in range(TT):
                psv = ps_sm_pool.tile([128, 128], DT.float32, tag="sm", name="psv")
                nc.tensor.matmul(
                    psv,
                    lhsT=ht_in[:, bass.ts(tt, 128)],
                    rhs=wv_sb,
                    start=True, stop=True,
                )
                vt = vpool.tile([128, H, 32], DT.bfloat16, tag="v", name="vt")
                nc.vector.tensor_copy(
                    vt[:, :, 0:DK], psv.rearrange("p (h v) -> p h v", h=H)
                )
                nc.vector.memset(vt[:, :, DK : DK + 1], 1.0)
                nc.vector.memset(vt[:, :, DK + 1 : 32], 0.0)
                v_tiles.append(vt)
            return m_in, qt_g, kt_g, v_tiles

        def emit_qc(state, b, qc):
            m_in, qt_g, kt_g, v_tiles = state
            qs = bass.ts(qc, 512)
            av_ps = [
                ps_av_pool.tile([128, 512], DT.float32, tag="av", name=f"av{r}")
                for r in range(2)
            ]
            pending_av = None  # (em_tile, tt, rnd)

            def emit_av(pend):
                em_t, tt_, rnd_ = pend
                for g in range(4):
                    h = 4 * rnd_ + g
                    nc.tensor.matmul(
                        av_ps[rnd_][32 * g : 32 * g + 32, :],
                        lhsT=v_tiles[tt_][:, h, :],
                        rhs=em_t[:, bass.ts(g, 512)],
                        start=(tt_ == 0),
                        stop=(tt_ == TT - 1),
                        tile_position=(0, 32 * g),
                        skip_group_check=True,
                    )

            for tt in range(TT):
                for rnd in range(2):
                    em_t = empool.tile([128, 2048], DT.bfloat16, tag="em", name="em_t")
                    e_halves = []
                    for sub in range(2):
                        ps_s = ps_s_pool.tile(
                            [128, 1024], DT.float32, tag=f"s{sub}", name=f"ps_s{sub}"
                        )
                        for gg in range(2):
                            g = 2 * sub + gg
                            kb_, ke_ = kt_g[rnd]
                            qb_, qe_ = qt_g[rnd]
                            psl = (slice(32 * g, 32 * g + DK), bass.ts(tt, 128))
                            qsl = (slice(32 * g, 32 * g + DK), qs)
                            for i, (lh, rh) in enumerate(
                                ((kb_, qb_), (kb_, qe_), (ke_, qb_))
                            ):
                                nc.tensor.matmul(
                                    ps_s[:, bass.ts(gg, 512)],
                                    lhsT=lh[psl],
                                    rhs=rh[qsl],
                                    start=(i == 0),
                                    stop=(i == 2),
                                    tile_position=(32 * g, 0),
                                    skip_group_check=True,
                                )
                        e_t = epool.tile(
                            [128, 1024], DT.bfloat16, tag=f"e{sub}", name=f"e_t{sub}"
                        )
                        nc.scalar.activation(
                            e_t, ps_s, mybir.ActivationFunctionType.Exp
                        )
                        e_halves.append(e_t)
                    if pending_av is not None:
                        emit_av(pending_av)
                    for sub in range(2):
                        for gg in range(2):
                            g = 2 * sub + gg
                            nc.vector.tensor_tensor(
                                em_t[:, bass.ts(g, 512)],
                                e_halves[sub][:, bass.ts(gg, 512)],
                                m_in[:, tt, qs],
                                mybir.AluOpType.mult,
                            )
                    pending_av = (em_t, tt, rnd)
            emit_av(pending_av)

            # ---- epilogue: normalize + output projection ----
            heads_f = epi.tile([128, 512], DT.float32, tag="heads", name="heads_f")
            srep_ps = ps_sm_pool.tile([128, 512], DT.float32, tag="sm", name="srep_ps")
            for rnd in range(2):
                av_sb = epi.tile([128, 512], DT.float32, tag="avsb", name="av_sb")
                nc.vector.tensor_copy(av_sb, av_ps[rnd])
                for g in range(4):
                    nc.sync.dma_start(
                        heads_f[64 * rnd + 16 * g : 64 * rnd + 16 * g + 16, :],
                        av_sb[32 * g : 32 * g + 16, :],
                    )
                nc.tensor.matmul(
                    srep_ps[64 * rnd : 64 * rnd + 64, :],
                    lhsT=sels_sb[:, rnd, :],
                    rhs=av_sb,
                    start=True,
                    stop=True,
                    tile_position=(0, 64 * rnd),
                )
            r_rep = epi.tile([128, 512], DT.float32, tag="rrep", name="r_rep")
            nc.vector.reciprocal(r_rep, srep_ps)
            heads_bf = epi.tile([128, 512], DT.float32r, tag="hbf", name="heads_bf")
            nc.vector.tensor_tensor(
                heads_bf, heads_f, r_rep, mybir.AluOpType.mult
            )
            for sub in range(4):
                ps_o = ps_sm_pool.tile([128, 128], DT.float32, tag="sm", name="ps_o")
                nc.tensor.matmul(
                    ps_o,
                    lhsT=heads_bf[:, bass.ts(sub, 128)],
                    rhs=wo_sb,
                    start=True,
                    stop=True,
                )
                o_sb = opool.tile([128, EMB], DT.float32, tag="o", name="o_sb")
                nc.vector.tensor_copy(o_sb, ps_o)
                r0 = qc * 512 + sub * 128
                nc.sync.dma_start(out[b, r0 : r0 + 128, :], o_sb)

        n_total = BL * repeat
        state = emit_proj(emit_load(0))
        for bb in range(n_total):
            b = bb % BL
            cur = state
            emit_qc(cur, b, 0)
            if bb + 1 < n_total:
                state = emit_proj(emit_load((bb + 1) % BL))
            emit_qc(cur, b, 1)

    nc.compile()
    return nc


_NC = {}


def _get_nc(repeat=1, score_dt=None):
    key = (repeat, str(score_dt))
    if key not in _NC:
        _NC[key] = _build_kernel(repeat, score_dt)
    return _NC[key]


def _prep_host(q, h, mask, w_query, w_key, w_value, w_out):
    """Host-side layout prep shared by all cores (pure numpy)."""
    qT = np.ascontiguousarray(np.asarray(q, np.float32).transpose(0, 2, 1))
    hT = np.ascontiguousarray(np.asarray(h, np.float32).transpose(0, 2, 1))
    # maskf^T[t, q]: 1.0 where attention is allowed
    mT = np.ascontiguousarray((~mask).transpose(0, 2, 1)).astype(BF16)

    def grp_pack(w, scale):
        # [H, EMB, DK] -> [EMB, 2, 128], head 4*grp+g at cols 32g..32g+16
        packed = np.zeros((EMB, 2, 128), np.float32)
        for grp in range(2):
            for g in range(4):
                packed[:, grp, 32 * g : 32 * g + DK] = w[4 * grp + g] * scale
        return packed

    wq = grp_pack(np.asarray(w_query, np.float32), SCALE)
    wk = grp_pack(np.asarray(w_key, np.float32), 1.0)
    wv = (
        np.asarray(w_value, np.float32)
        .transpose(1, 0, 2)
        .reshape(EMB, H * DK)
        .copy()
    )
    wo = np.asarray(w_out, np.float32).reshape(H * DK, EMB).copy()
    sels = np.zeros((2, 128, 64), np.float32)
    for g in range(4):
        sels[:, 32 * g + 16, 16 * g : 16 * g + 16] = 1.0
    return qT, hT, mT, wq, wk, wv, wo, sels


def kernel(q, h, mask, w_query, w_key, w_value, w_out):
    nc = _get_nc()
    qT, hT, mT, wq, wk, wv, wo, sels = _prep_host(
        q, h, mask, w_query, w_key, w_value, w_out
    )
    in_maps = []
    for c in range(N_CORES):
        sl = slice(c * BL, (c + 1) * BL)
        in_maps.append(
            {
                "qt": qT[sl],
                "ht": hT[sl],
                "mt": mT[sl],
                "wq": wq,
                "wk": wk,
                "wv": wv,
                "wo": wo,
                "sels": sels,
            }
        )
    res = run_bass_kernel_spmd(nc, in_maps, core_ids=list(range(N_CORES)))
    return np.concatenate([r["out"] for r in res.results], axis=0)


# revision 27
# speedup vs baseline: 1.4176x; 1.4176x over previous
"""Multi-head attention Trainium2 kernel (nn_MultiHeadAttention_489626272000).

Strategy
--------
Data-parallel over batch: 32 batches -> 8 NeuronCores, 4 batches/core.

Per batch, everything is computed in a "transposed score" orientation
S^T[t, q] so that
  * the score matmul S^T = K_h^T-major is a K=16 contraction packed 4 heads
    at a time into PE row-groups (tile_position=(32g, 0)),
  * exp() runs on ScalarE straight out of PSUM (4 heads = 4 banks = one
    N=2048 ACTIVATE),
  * the mask is a plain elementwise multiply on VectorE (bf16, 2x mode),
  * the AV matmul consumes Ẽ^T directly (contraction over t = partitions),
    packed 4 heads into PE col-groups (tile_position=(0, 32g)), with a
    ones-column appended to V so the softmax denominator falls out of the
    same matmul,
  * softmax normalization happens once per [hv=128, q] tile at the end
    (exact-f32 sums via partition-gather DMAs + one VectorE reciprocal).

Host-side prep (inside kernel()): transpose q/h/mask per-batch and cast to
bf16; pack weights into PE-friendly group layouts. The device kernel then
needs zero on-chip transposes.

No max-subtraction is needed before exp: scores have sigma≈2.7, so the max
over all 268M scores is ~15 -> exp() <= ~3e6, far inside fp32/bf16 range.
"""

import sys

for _p in ("/opt/trn_rl_repo",):
    if _p not in sys.path:
        sys.path.insert(0, _p)

from contextlib import ExitStack

import ml_dtypes
import numpy as np

import concourse.bass as bass
import concourse.tile as tile
from concourse import bacc, mybir
from concourse.bass_utils import run_bass_kernel_spmd

BF16 = ml_dtypes.bfloat16
F32 = np.float32

N_CORES = 8
B, NQ, T, EMB = 32, 1024, 1024, 128
H, DK = 8, 16
BL = B // N_CORES  # batches per core
SCALE = 1.0 / np.sqrt(DK)  # folded into w_query on the host

TT = T // 128  # 8 t-tiles per batch
QC = NQ // 512  # 2 q-chunks per batch
DT = mybir.dt


def _build_kernel(repeat=1, score_dt=None):
    nc = bacc.Bacc("TRN2", target_bir_lowering=False, debug=False)

    qT = nc.dram_tensor("qt", [BL, EMB, NQ], DT.float16, kind="ExternalInput")
    hT = nc.dram_tensor("ht", [BL, EMB, T], DT.float16, kind="ExternalInput")
    mT = nc.dram_tensor("mt", [BL, T, NQ], DT.float16, kind="ExternalInput")
    # wq/wk: [emb, grp, 128] where col 32*g+v holds head (4*grp+g) dim v
    # (cols 32g+16..32g+32 are zero padding so heads align to PE row groups)
    wq = nc.dram_tensor("wq", [EMB, 2, 128], DT.float16, kind="ExternalInput")
    wk = nc.dram_tensor("wk", [EMB, 2, 128], DT.float16, kind="ExternalInput")
    # wv: [emb, h*16+v] plain packing; wo: [h*16+v, emb]
    wv = nc.dram_tensor("wv", [EMB, 128], DT.float16, kind="ExternalInput")
    wo = nc.dram_tensor("wo", [128, EMB], DT.float16, kind="ExternalInput")
    # sums-replication selector: s_rep[64*rnd + 16g + v, q] = av_rnd[32g + 16, q]
    sels = nc.dram_tensor("sels", [2, 128, 64], DT.float32, kind="ExternalInput")
    out = nc.dram_tensor("out", [BL, NQ, EMB], DT.float32, kind="ExternalOutput")

    if score_dt is None:
        score_dt = DT.float32r
    with tile.TileContext(nc) as tc, ExitStack() as ctx:
        # ---- pools ----
        singles = ctx.enter_context(tc.tile_pool(name="singles", bufs=1))
        inp = ctx.enter_context(tc.tile_pool(name="inp", bufs=2))
        qk = ctx.enter_context(tc.tile_pool(name="qk", bufs=8))
        vpool = ctx.enter_context(tc.tile_pool(name="v", bufs=16))
        epool = ctx.enter_context(tc.tile_pool(name="e", bufs=3))
        empool = ctx.enter_context(tc.tile_pool(name="em", bufs=3))
        epi = ctx.enter_context(tc.tile_pool(name="epi", bufs=2))
        opool = ctx.enter_context(tc.tile_pool(name="o", bufs=4))

        ps_s_pool = ctx.enter_context(tc.tile_pool(name="pss", bufs=1, space="PSUM"))
        ps_av_pool = ctx.enter_context(tc.tile_pool(name="psav", bufs=2, space="PSUM"))
        ps_sm_pool = ctx.enter_context(tc.tile_pool(name="pssm", bufs=2, space="PSUM"))

        # ---- constants ----
        wq_sb = singles.tile([EMB, 2, 128], DT.float16)
        nc.sync.dma_start(wq_sb, wq[:])
        wk_sb = singles.tile([EMB, 2, 128], DT.float16)
        nc.sync.dma_start(wk_sb, wk[:])
        wv_sb = singles.tile([EMB, 128], DT.float16)
        nc.sync.dma_start(wv_sb, wv[:])
        wo_sb = singles.tile([128, EMB], DT.float16)
        nc.sync.dma_start(wo_sb, wo[:])
        sels_sb = singles.tile([128, 2, 64], DT.float32)
        nc.sync.dma_start(sels_sb, sels[:].rearrange("r p m -> p r m"))
        expbias = singles.tile([128, 1], DT.float32)
        nc.vector.memset(expbias, -15.0)

        def emit_load(b):
            qt_in = inp.tile([128, NQ], DT.float16, tag="qt", name="qt_in")
            nc.sync.dma_start(qt_in, qT[b])
            ht_in = inp.tile([128, T], DT.float16, tag="ht", name="ht_in")
            nc.sync.dma_start(ht_in, hT[b])
            m_in = inp.tile([128, TT, NQ], DT.float16, tag="mt", name="m_in")
            nc.sync.dma_start(m_in, mT[b].rearrange("(tt p) q -> p tt q", p=128))
            return qt_in, ht_in, m_in

        def emit_proj(loaded):
            qt_in, ht_in, m_in = loaded
            qt_g, kt_g = [], []
            for grp in range(2):
                qg = qk.tile([128, NQ], DT.float16, tag="qk", name="qg")
                kg = qk.tile([128, T], DT.float16, tag="qk", name="kg")
                for half in range(2):
                    sl = bass.ts(half, 512)
                    psq = ps_sm_pool.tile([128, 512], DT.float32, tag="sm", name="psq")
                    nc.tensor.matmul(
                        psq,
                        lhsT=wq_sb[:, grp, :],
                        rhs=qt_in[:, sl],
                        start=True, stop=True,
                    )
                    nc.vector.tensor_copy(qg[:, sl], psq)
                    psk = ps_sm_pool.tile([128, 512], DT.float32, tag="sm", name="psk")
                    nc.tensor.matmul(
                        psk,
                        lhsT=wk_sb[:, grp, :],
                        rhs=ht_in[:, sl],
                        start=True, stop=True,
                    )
                    nc.vector.tensor_copy(kg[:, sl], psk)
                qt_g.append(qg)
                kt_g.append(kg)

            v_tiles = []
            for tt in range(TT):
                psv = ps_sm_pool.tile([128, 128], DT.float32, tag="sm", name="psv")
                nc.tensor.matmul(
                    psv,
                    lhsT=ht_in[:, bass.ts(tt, 128)],
                    rhs=wv_sb,
                    start=True, stop=True,
                )
                vt = vpool.tile([128, H, 32], DT.float16, tag="v", name="vt")
                nc.vector.tensor_copy(
                    vt[:, :, 0:DK], psv.rearrange("p (h v) -> p h v", h=H)
                )
                nc.vector.memset(vt[:, :, DK : DK + 1], 1.0)
                nc.vector.memset(vt[:, :, DK + 1 : 32], 0.0)
                v_tiles.append(vt)
            return m_in, qt_g, kt_g, v_tiles

        def emit_qc(state, b, qc):
            m_in, qt_g, kt_g, v_tiles = state
            qs = bass.ts(qc, 512)
            av_ps = [
                ps_av_pool.tile([128, 512], DT.float32, tag="av", name=f"av{r}")
                for r in range(2)
            ]
            pending_av = None  # (em_tile, tt, rnd)

            def emit_av(pend):
                em_t, tt_, rnd_ = pend
                for g in range(4):
                    h = 4 * rnd_ + g
                    nc.tensor.matmul(
                        av_ps[rnd_][32 * g : 32 * g + 32, :],
                        lhsT=v_tiles[tt_][:, h, :],
                        rhs=em_t[:, bass.ts(g, 512)],
                        start=(tt_ == 0),
                        stop=(tt_ == TT - 1),
                        tile_position=(0, 32 * g),
                        skip_group_check=True,
                    )

            for tt in range(TT):
                for rnd in range(2):
                    em_t = empool.tile([128, 2048], DT.float16, tag="em", name="em_t")
                    e_halves = []
                    for sub in range(2):
                        ps_s = ps_s_pool.tile(
                            [128, 1024], DT.float32, tag=f"s{sub}", name=f"ps_s{sub}"
                        )
                        for gg in range(2):
                            g = 2 * sub + gg
                            nc.tensor.matmul(
                                ps_s[:, bass.ts(gg, 512)],
                                lhsT=kt_g[rnd][
                                    32 * g : 32 * g + DK, bass.ts(tt, 128)
                                ],
                                rhs=qt_g[rnd][32 * g : 32 * g + DK, qs],
                                start=True,
                                stop=True,
                                tile_position=(32 * g, 0),
                            )
                        e_t = epool.tile(
                            [128, 1024], DT.float16, tag=f"e{sub}", name=f"e_t{sub}"
                        )
                        nc.scalar.activation(
                            e_t, ps_s, mybir.ActivationFunctionType.Exp, bias=expbias
                        )
                        e_halves.append(e_t)
                    if pending_av is not None:
                        emit_av(pending_av)
                    for sub in range(2):
                        for gg in range(2):
                            g = 2 * sub + gg
                            nc.vector.tensor_tensor(
                                em_t[:, bass.ts(g, 512)],
                                e_halves[sub][:, bass.ts(gg, 512)],
                                m_in[:, tt, qs],
                                mybir.AluOpType.mult,
                            )
                    pending_av = (em_t, tt, rnd)
            emit_av(pending_av)

            # ---- epilogue: normalize + output projection ----
            heads_f = epi.tile([128, 512], DT.float32, tag="heads", name="heads_f")
            srep_ps = ps_sm_pool.tile([128, 512], DT.float32, tag="sm", name="srep_ps")
            for rnd in range(2):
                av_sb = epi.tile([128, 512], DT.float32, tag="avsb", name="av_sb")
                nc.vector.tensor_copy(av_sb, av_ps[rnd])
                for g in range(4):
                    nc.sync.dma_start(
                        heads_f[64 * rnd + 16 * g : 64 * rnd + 16 * g + 16, :],
                        av_sb[32 * g : 32 * g + 16, :],
                    )
                nc.tensor.matmul(
                    srep_ps[64 * rnd : 64 * rnd + 64, :],
                    lhsT=sels_sb[:, rnd, :],
                    rhs=av_sb,
                    start=True,
                    stop=True,
                    tile_position=(0, 64 * rnd),
                )
            r_rep = epi.tile([128, 512], DT.float32, tag="rrep", name="r_rep")
            nc.vector.reciprocal(r_rep, srep_ps)
            heads_bf = epi.tile([128, 512], DT.float16, tag="hbf", name="heads_bf")
            nc.vector.tensor_tensor(
                heads_bf, heads_f, r_rep, mybir.AluOpType.mult
            )
            for sub in range(4):
                ps_o = ps_sm_pool.tile([128, 128], DT.float32, tag="sm", name="ps_o")
                nc.tensor.matmul(
                    ps_o,
                    lhsT=heads_bf[:, bass.ts(sub, 128)],
                    rhs=wo_sb,
                    start=True,
                    stop=True,
                )
                o_sb = opool.tile([128, EMB], DT.float32, tag="o", name="o_sb")
                nc.vector.tensor_copy(o_sb, ps_o)
                r0 = qc * 512 + sub * 128
                nc.sync.dma_start(out[b, r0 : r0 + 128, :], o_sb)

        n_total = BL * repeat
        state = emit_proj(emit_load(0))
        for bb in range(n_total):
            b = bb % BL
            cur = state
            emit_qc(cur, b, 0)
            if bb + 1 < n_total:
                state = emit_proj(emit_load((bb + 1) % BL))
            emit_qc(cur, b, 1)

    nc.compile()
    return nc


_NC = {}


def _get_nc(repeat=1, score_dt=None):
    key = (repeat, str(score_dt))
    if key not in _NC:
        _NC[key] = _build_kernel(repeat, score_dt)
    return _NC[key]


def _prep_host(q, h, mask, w_query, w_key, w_value, w_out):
    """Host-side layout prep shared by all cores (pure numpy)."""
    qT = np.ascontiguousarray(np.asarray(q, np.float32).transpose(0, 2, 1)).astype(np.float16)
    hT = np.ascontiguousarray(np.asarray(h, np.float32).transpose(0, 2, 1)).astype(np.float16)
    # maskf^T[t, q]: 1.0 where attention is allowed
    mT = np.ascontiguousarray((~mask).transpose(0, 2, 1)).astype(np.float16)

    def grp_pack(w, scale):
        # [H, EMB, DK] -> [EMB, 2, 128], head 4*grp+g at cols 32g..32g+16
        packed = np.zeros((EMB, 2, 128), np.float32)
        for grp in range(2):
            for g in range(4):
                packed[:, grp, 32 * g : 32 * g + DK] = w[4 * grp + g] * scale
        return packed.astype(np.float16)

    wq = grp_pack(np.asarray(w_query, np.float32), SCALE)
    wk = grp_pack(np.asarray(w_key, np.float32), 1.0)
    wv = (
        np.asarray(w_value, np.float32)
        .transpose(1, 0, 2)
        .reshape(EMB, H * DK)
        .astype(np.float16)
    )
    wo = np.asarray(w_out, np.float32).reshape(H * DK, EMB).astype(np.float16)
    sels = np.zeros((2, 128, 64), np.float32)
    for g in range(4):
        sels[:, 32 * g + 16, 16 * g : 16 * g + 16] = 1.0
    return qT, hT, mT, wq, wk, wv, wo, sels


def kernel(q, h, mask, w_query, w_key, w_value, w_out):
    nc = _get_nc()
    qT, hT, mT, wq, wk, wv, wo, sels = _prep_host(
        q, h, mask, w_query, w_key, w_value, w_out
    )
    in_maps = []
    for c in range(N_CORES):
        sl = slice(c * BL, (c + 1) * BL)
        in_maps.append(
            {
                "qt": qT[sl],
                "ht": hT[sl],
                "mt": mT[sl],
                "wq": wq,
                "wk": wk,
                "wv": wv,
                "wo": wo,
                "sels": sels,
            }
        )
    res = run_bass_kernel_spmd(nc, in_maps, core_ids=list(range(N_CORES)))
    return np.concatenate([r["out"] for r in res.results], axis=0)
